# revision 1
# baseline (speedup 1.0000x reference)
"""Octree2Col (out[c,k,i] = data_in[c, neigh[i,k]] masked) on 8 TRN2 cores.

Sharding: node dim N=262144 split 8 ways (32768 nodes/core); every core
gets the full feature table (all-gather-on-host of data_in, 33 MB) in a
transposed [N+1, 32] layout whose last row is zeros, so invalid (-1)
neighbors are redirected to the zero row and need no masking pass.

Per core, per 128-node block: 27 indirect DMA gathers (one per kernel
position k; each supplies 128 int32 row indices, one per partition) fill
gt[128, 864] with gt[p, k*32+c] = table[idx[p,k], c]; the PE transposes
128-column chunks into PSUM; the DVE copies them into per-(k,c) node
strips; strips are DMA'd to the [32, 27, 32768] output with a
(k,c)->(c,k) reordering access pattern.
"""
import numpy as np

C = 32
K = 27
N = 262144
P = 128
KC = K * C          # 864
NCHUNK = 7          # ceil(864 / 128)
NCORES = 8
NODES_PER_CORE = N // NCORES
STRIP = 2048

_MAX_WAITS = 1


def _split_waits_json(raw: bytes) -> bytes:
    """This walrus build rejects instructions carrying more than a couple
    of sync waits; hoist excess waits onto same-engine NoOps inserted just
    before the instruction (engine program order = block list order)."""
    import json

    j = json.loads(raw)
    counter = [0]
    changed = False
    for fn in j.get("functions", []):
        for bb in fn.get("blocks", []):
            out = []
            for ins in bb.get("instructions", []):
                si = ins.get("sync_info")
                waits = (si or {}).get("on_wait") or []
                if len(waits) > _MAX_WAITS:
                    changed = True
                    keep = waits[:_MAX_WAITS]
                    extra = waits[_MAX_WAITS:]
                    for w in extra:
                        counter[0] += 1
                        out.append(
                            {
                                "debug": ins.get("debug", 0),
                                "engine": ins["engine"],
                                "ins": [],
                                "name": f"I-wsplit-{counter[0]}",
                                "opcode": "NoOp",
                                "outs": [],
                                "sync_info": {"on_update": [], "on_wait": [w]},
                                "text_hint": "wait_split",
                            }
                        )
                    si["on_wait"] = keep
                out.append(ins)
            bb["instructions"] = out
    if not changed:
        return raw
    return json.dumps(j).encode()


def _apply_patches():
    import concourse.bass as bass
    import concourse.tile as tile
    from concourse.vector_clock import ScopedClock

    def _drain_and_barrier(self, tick_clock, wait_clock):
        vc = tick_clock.global_clock
        for proc in range(len(vc)):
            t = vc[proc]
            if t > 0:
                sub = ScopedClock()
                sub.require_at_least(None, proc, t)
                nop_inst = self.nc.sync.nop(nofuse=True, hint=f"drain_wait_{proc}")
                wait_clock.add_sem_waits(nop_inst.ins, sub)
        self.nc.sync.drain()
        self.nc.all_engine_barrier()
        assert self.sems is not None
        popped = self.nc._tile_sem_poison_stack.pop()
        assert popped is self._sem_poison
        self.nc.clear_and_free_semaphores(list(self.sems.allocated().values()))
        self.nc.all_engine_barrier()

    tile.TileContext._drain_and_barrier = _drain_and_barrier

    if not getattr(bass.Bass, "_wait_split_patched", False):
        orig = bass.Bass.to_json_bytes

        def to_json_bytes(self, *a, **kw):
            return _split_waits_json(orig(self, *a, **kw))

        bass.Bass.to_json_bytes = to_json_bytes
        bass.Bass._wait_split_patched = True


def build(nodes_per_core: int = NODES_PER_CORE, strip_nodes: int = STRIP):
    _apply_patches()
    import concourse.bass as bass
    import concourse.mybir as mybir
    import concourse.tile as tile
    from concourse.masks import make_identity

    assert nodes_per_core % strip_nodes == 0
    n_strips = nodes_per_core // strip_nodes
    blocks_per_strip = strip_nodes // P

    nc = bass.Bass()
    table = nc.declare_dram_parameter(
        "table", [N + 1, C], mybir.dt.float32, isOutput=False
    )
    idx = nc.declare_dram_parameter(
        "idx", [nodes_per_core, K], mybir.dt.int32, isOutput=False
    )
    out = nc.declare_dram_parameter(
        "out", [C, K, nodes_per_core], mybir.dt.float32, isOutput=True
    )

    with tile.TileContext(nc) as tc:
        with (
            tc.tile_pool(name="const", bufs=1) as cpool,
            tc.tile_pool(name="gather", bufs=3) as gpool,
            tc.tile_pool(name="idxp", bufs=3) as ipool,
            tc.tile_pool(name="strips", bufs=2) as spool,
            tc.tile_pool(name="psum", bufs=8, space="PSUM") as ppool,
        ):
            ident = cpool.tile([P, P], mybir.dt.float32, name="ident")
            make_identity(nc, ident[:])

            for s in range(n_strips):
                strips = [
                    spool.tile(
                        [P, strip_nodes],
                        mybir.dt.float32,
                        tag=f"st{ch}",
                        name=f"strip{ch}",
                    )
                    for ch in range(NCHUNK)
                ]
                for b in range(blocks_per_strip):
                    node0 = s * strip_nodes + b * P
                    idxt = ipool.tile([P, K], mybir.dt.int32, name="idxt")
                    nc.sync.dma_start(out=idxt[:], in_=idx[node0 : node0 + P, :])
                    gt = gpool.tile([P, NCHUNK * P], mybir.dt.float32, name="gt")
                    for k in range(K):
                        nc.gpsimd.indirect_dma_start(
                            out=gt[:, k * C : (k + 1) * C],
                            out_offset=None,
                            in_=table[:],
                            in_offset=bass.IndirectOffsetOnAxis(
                                ap=idxt[:, k : k + 1], axis=0
                            ),
                        )
                    for ch in range(NCHUNK):
                        pt = ppool.tile(
                            [P, P], mybir.dt.float32, tag="pt", name="pt"
                        )
                        nc.tensor.transpose(
                            out=pt[:],
                            in_=gt[:, ch * P : (ch + 1) * P],
                            identity=ident[:],
                        )
                        rows = P if ch < NCHUNK - 1 else KC - (NCHUNK - 1) * P
                        nc.vector.tensor_copy(
                            out=strips[ch][:rows, b * P : (b + 1) * P],
                            in_=pt[:rows, :],
                        )
                for ch in range(NCHUNK):
                    k0 = ch * 4
                    nk = 4 if ch < NCHUNK - 1 else K - k0
                    dst = out[:, k0 : k0 + nk, s * strip_nodes : (s + 1) * strip_nodes]
                    dst_t = dst.rearrange("c k n -> k c n")
                    nc.sync.dma_start(out=dst_t, in_=strips[ch][: nk * C, :])
    return nc


def prep_inputs(data_in: np.ndarray, neigh: np.ndarray):
    table = np.empty((N + 1, C), dtype=np.float32)
    table[:N] = np.ascontiguousarray(np.asarray(data_in, dtype=np.float32).T)
    table[N] = 0.0
    idx = np.where(np.asarray(neigh) >= 0, np.asarray(neigh), N).astype(np.int32)
    return table, np.ascontiguousarray(idx)


def kernel(data_in: np.ndarray, neigh: np.ndarray) -> np.ndarray:
    from concourse.bass_utils import run_bass_kernel_spmd

    table, idx = prep_inputs(data_in, neigh)
    nc = build()
    in_maps = [
        {
            "table": table,
            "idx": idx[d * NODES_PER_CORE : (d + 1) * NODES_PER_CORE],
        }
        for d in range(NCORES)
    ]
    res = run_bass_kernel_spmd(nc, in_maps, core_ids=list(range(NCORES)))
    return np.concatenate([r["out"] for r in res.results], axis=2)
